# revision 1
# baseline (speedup 1.0000x reference)
"""Trainium2 Bass kernel for nn_MixtureOfRoutingAttention.

Strategy: data-parallel over B=8 (one sample per NeuronCore). The top-1
routing argmax is computed on host (it only decides dispatch); the selected
expert weight stacks are gathered per sample on host and shipped pre-transposed
to each core. Everything x-dependent (LayerNorms, QKV/out projections, three
attentions, MLP) runs on device in fp32 using the fp32r fast matmul path.

Device-side layout: activations are kept feature-major (x^T: [D, T], feature on
partitions) so every GEMM contracts over the partition axis with zero on-chip
transposes. Attention uses transposed scores S^T[j, i] so softmax
normalization is deferred: a ones-column appended to V yields the softmax
denominators as row HD of the PV psum accumulation.
"""

import math
import os
from contextlib import ExitStack

import numpy as np

import concourse.bass as bass
import concourse.bacc as bacc
import concourse.tile as tile
import concourse.mybir as mybir
from concourse import bass_utils

dt = mybir.dt
AF = mybir.ActivationFunctionType
ALU = mybir.AluOpType

P = 128
T = 512
D = 768
H = 8
HD = 96
DFF = 3072
NCORES = 8
ND = D // P  # 6
NT = T // P  # 4
NF = DFF // P  # 24
SCALE = 1.0 / math.sqrt(HD)
EPS = 1e-5
F32 = dt.float32


def _r(ap):
    return ap.bitcast(dt.float32r)


def _bcast_ap(src_1d, parts=P):
    """Partition-broadcast DMA access pattern for a 1-D DRAM AP."""
    return bass.AP(
        tensor=src_1d.tensor,
        offset=src_1d.offset,
        ap=[[0, parts], list(src_1d.ap[0])],
    )


PHASE_MARKS = []


def build(repeat: int = 1):
    PHASE_MARKS.clear()
    nc = bacc.Bacc(
        "TRN2",
        target_bir_lowering=False,
        debug=False,
        enable_asserts=False,
        num_devices=NCORES,
    )

    def din(name, shape):
        return nc.dram_tensor(name, shape, F32, kind="ExternalInput").ap()

    xT_d = din("xT", [D, T])
    ones_d = din("ones", [P, 33])
    diag_d = din("diag", [P, P])
    ln_d = {n: din(n, [D]) for n in ("g_s", "b_s", "g_t", "b_t", "g_m", "b_m")}

    spWqP_d = din("spWqP", [H, P, ND * HD])
    spWkP_d = din("spWkP", [H, P, ND * HD])
    spWvP_d = din("spWvP", [ND, P, D])
    spBqkv_d = din("spBqkv", [3 * D])
    spWoP_d = din("spWoP", [ND, HD, H * P])
    spBo_d = din("spBo", [D])

    tpWqP_d = din("tpWqP", [H, P, ND * HD])
    tpWkP_d = din("tpWkP", [H, P, ND * HD])
    tpWvP_d = din("tpWvP", [ND, P, D])
    tpBq_d = din("tpBq", [D])
    tpBk_d = din("tpBk", [D])
    tpBv_d = din("tpBv", [D])
    tpWoP_d = din("tpWoP", [ND, HD, H * P])
    tpBo_d = din("tpBo", [D])

    cWqP_d = din("cWqP", [H, P, ND * HD])
    cWkP_d = din("cWkP", [H, P, ND * HD])
    cWvP_d = din("cWvP", [ND, P, D])
    cBqkv_d = din("cBqkv", [3 * D])
    cWoP_d = din("cWoP", [ND, HD, H * P])
    cBo_d = din("cBo", [D])

    mW1P_d = din("mW1P", [NF, P, ND * P])
    mB1_d = din("mB1", [DFF])
    mW2T_d = din("mW2T", [DFF, D])
    mB2_d = din("mB2", [D])

    outT_d = nc.dram_tensor("outT", [D, T], F32, kind="ExternalOutput").ap()

    with tile.TileContext(nc) as tc, ExitStack() as ctx:
        ctx.enter_context(
            nc.allow_low_precision(reason="float32r views of fp32 data")
        )
        const = ctx.enter_context(tc.tile_pool(name="const", bufs=1))
        big = ctx.enter_context(tc.tile_pool(name="big", bufs=1))
        wq = ctx.enter_context(tc.tile_pool(name="wq", bufs=6))
        wv = ctx.enter_context(tc.tile_pool(name="wv", bufs=3))
        wo = ctx.enter_context(tc.tile_pool(name="wo", bufs=2))
        w2 = ctx.enter_context(tc.tile_pool(name="w2", bufs=2))
        tmp = ctx.enter_context(tc.tile_pool(name="tmp", bufs=3))
        ex = ctx.enter_context(tc.tile_pool(name="ex", bufs=6))
        qk = ctx.enter_context(tc.tile_pool(name="qk", bufs=4))
        rows = ctx.enter_context(tc.tile_pool(name="rows", bufs=4))
        ps = ctx.enter_context(tc.tile_pool(name="ps", bufs=8, space="PSUM"))

        def pst(nm):
            return ps.tile([P, T], F32, name=nm, tag="ps", bufs=8)

        # ---- constants / params (loaded once, outside any repeat loop) ----
        ones_sb = const.tile([P, 33], F32, name="ones_sb", tag="ones")
        nc.sync.dma_start(out=_r(ones_sb), in_=_r(ones_d))
        diag_sb = const.tile([P, P], F32, name="diag_sb", tag="diag")
        nc.sync.dma_start(out=diag_sb, in_=diag_d)
        epsc = const.tile([P, 1], F32, name="epsc", tag="eps")
        nc.vector.memset(epsc, EPS)
        zeros_sb = const.tile([P, T], F32, name="zeros_sb", tag="zeros")
        nc.vector.memset(zeros_sb, 0.0)

        ln_sb = {}
        for n in ln_d:
            t = const.tile([P, ND], F32, name=f"ln_{n}", tag=f"ln_{n}")
            nc.sync.dma_start(out=t, in_=ln_d[n].rearrange("(a p) -> p a", p=P))
            ln_sb[n] = t

        def load_bias96(name, src_ap):
            t = const.tile([HD, H], F32, name=name, tag=name)
            nc.sync.dma_start(out=t, in_=src_ap.rearrange("(h k) -> k h", k=HD))
            return t

        spBq96 = load_bias96("spBq96", spBqkv_d[0:D])
        spBk96 = load_bias96("spBk96", spBqkv_d[D : 2 * D])
        tpBq96 = load_bias96("tpBq96", tpBq_d)
        tpBk96 = load_bias96("tpBk96", tpBk_d)
        cBq96 = load_bias96("cBq96", cBqkv_d[0:D])
        cBk96 = load_bias96("cBk96", cBqkv_d[D : 2 * D])

        def load_bcast(name, src_1d):
            t = const.tile([P, D], F32, name=name, tag=name)
            nc.gpsimd.dma_start(out=t, in_=_bcast_ap(src_1d))
            return t

        spBv_bc = load_bcast("spBv_bc", spBqkv_d[2 * D : 3 * D])
        tpBv_bc = load_bcast("tpBv_bc", tpBv_d)
        cBv_bc = load_bcast("cBv_bc", cBqkv_d[2 * D : 3 * D])

        def load_colvec(name, src_1d, cols):
            t = const.tile([P, cols], F32, name=name, tag=name)
            nc.sync.dma_start(out=t, in_=src_1d.rearrange("(a p) -> p a", p=P))
            return t

        spBo_sb = load_colvec("spBo_sb", spBo_d, ND)
        tpBo_sb = load_colvec("tpBo_sb", tpBo_d, ND)
        cBo_sb = load_colvec("cBo_sb", cBo_d, ND)
        mB1_sb = load_colvec("mB1_sb", mB1_d, NF)
        mB2_sb = load_colvec("mB2_sb", mB2_d, ND)

        # ---- body helpers ----

        def ln_stats(src):
            """src: [P, ND, T] (f32r-written). Returns (meanb, rstdb) [P, T]."""
            ps_mean = pst("ps_mean")
            ps_sq = pst("ps_sq")
            for a in range(ND):
                sq = tmp.tile([P, T], F32, name="sq", tag="tmp")
                nc.vector.tensor_mul(_r(sq), src[:, a, :], src[:, a, :])
                nc.tensor.matmul(
                    ps_mean[0:1, :], _r(ones_sb[:, 0:1]), _r(src[:, a, :]),
                    start=(a == 0), stop=(a == ND - 1),
                )
                nc.tensor.matmul(
                    ps_sq[0:1, :], _r(ones_sb[:, 0:1]), _r(sq),
                    start=(a == 0), stop=(a == ND - 1),
                )
            mrow = rows.tile([P, T], F32, name="mrow", tag="rows")
            nc.vector.tensor_scalar_mul(mrow[0:1, :], ps_mean[0:1, :], 1.0 / D)
            srow = rows.tile([P, T], F32, name="srow", tag="rows")
            nc.vector.tensor_scalar_mul(srow[0:1, :], ps_sq[0:1, :], 1.0 / D)
            trow = rows.tile([P, T], F32, name="trow", tag="rows")
            nc.vector.tensor_mul(trow[0:1, :], mrow[0:1, :], mrow[0:1, :])
            nc.vector.tensor_sub(srow[0:1, :], srow[0:1, :], trow[0:1, :])
            nc.scalar.activation(
                trow[0:1, :], srow[0:1, :], AF.Sqrt, bias=epsc[0:1, :]
            )
            rrow = rows.tile([P, T], F32, name="rrow", tag="rows")
            nc.vector.reciprocal(rrow[0:1, :], trow[0:1, :])

            meanb = big.tile([P, T], F32, name="meanb", tag="meanb")
            nc.gpsimd.partition_broadcast(meanb, mrow[0:1, :])
            rstdb = big.tile([P, T], F32, name="rstdb", tag="rstdb")
            nc.gpsimd.partition_broadcast(rstdb, rrow[0:1, :])
            return meanb, rstdb

        def ln_apply(src, meanb, rstdb, outs):
            """outs: list of (dst [P, ND, T], gamma_sb, beta_sb)."""
            for a in range(ND):
                xc = tmp.tile([P, T], F32, name="xc", tag="tmp")
                nc.vector.tensor_sub(xc, src[:, a, :], meanb)
                nc.vector.tensor_mul(xc, xc, rstdb)
                for dst, g_sb, b_sb in outs:
                    nc.scalar.activation(
                        _r(dst[:, a, :]), xc, AF.Identity,
                        bias=b_sb[:, a : a + 1], scale=g_sb[:, a : a + 1],
                    )

        def gemm_head(src, wP_d, bias96, h, dst96, use_act=False):
            """dst96[0:HD, :] = (W[:, head h cols].T @ src) + bias."""
            wt = wq.tile([P, ND, P], F32, name="wt", tag="wq")
            nc.sync.dma_start(
                out=_r(wt[:, :, 0:HD]),
                in_=_r(wP_d[h].rearrange("p (a e) -> p a e", e=HD)),
            )
            pq = pst("pq")
            for a in range(ND):
                nc.tensor.matmul(
                    pq[0:HD, :], _r(wt[:, a, 0:HD]), _r(src[:, a, :]),
                    start=(a == 0), stop=(a == ND - 1),
                )
            if use_act:
                nc.scalar.activation(
                    _r(dst96[0:HD, :]), pq[0:HD, :], AF.Identity,
                    bias=bias96[:, h : h + 1],
                )
            else:
                nc.vector.tensor_scalar_add(
                    _r(dst96[0:HD, :]), pq[0:HD, :], bias96[:, h : h + 1]
                )

        def gemm_v_token(src, wP_d, vbias_bc, Vt):
            """Vt: [P, NT, H, HD+1] token-major V with trailing ones column."""
            for half, n in ((0, 512), (1, 256)):
                pvs = [pst(f"pv{t}") for t in range(NT)]
                for a in range(ND):
                    wvt = wv.tile([P, 512], F32, name="wvt", tag="wv")
                    nc.sync.dma_start(
                        out=_r(wvt[:, 0:n]),
                        in_=_r(wP_d[a, :, half * 512 : half * 512 + n]),
                    )
                    for t in range(NT):
                        nc.tensor.matmul(
                            pvs[t][:, 0:n],
                            _r(src[:, a, t * P : (t + 1) * P]),
                            _r(wvt[:, 0:n]),
                            start=(a == 0), stop=(a == ND - 1),
                        )
                for t in range(NT):
                    if half == 0:
                        nc.vector.tensor_add(
                            _r(Vt[:, t, 0:5, 0:HD]),
                            pvs[t][:, 0:480].rearrange("p (h k) -> p h k", k=HD),
                            vbias_bc[:, 0:480].rearrange("p (h k) -> p h k", k=HD),
                        )
                        nc.vector.tensor_add(
                            _r(Vt[:, t, 5, 0:32]),
                            pvs[t][:, 480:512],
                            vbias_bc[:, 480:512],
                        )
                    else:
                        nc.vector.tensor_add(
                            _r(Vt[:, t, 5, 32:HD]),
                            pvs[t][:, 0:64],
                            vbias_bc[:, 512:576],
                        )
                        nc.vector.tensor_add(
                            _r(Vt[:, t, 6:8, 0:HD]),
                            pvs[t][:, 64:256].rearrange("p (h k) -> p h k", k=HD),
                            vbias_bc[:, 576:768].rearrange("p (h k) -> p h k", k=HD),
                        )
            nc.vector.tensor_copy(
                _r(Vt[:, :, :, HD]),
                ones_sb[:, 1 : 1 + NT * H].rearrange("p (a h) -> p a h", h=H),
            )

        def attn_branch(src_q, src_k, Vt, attnT, causal,
                        wqP, bq96, wkP, bk96, heads=None):
            """Per-head fused q/k projection + attention."""
            for h in (heads if heads is not None else range(H)):
                qh = qk.tile([P, T], F32, name="qh", tag="qk")
                kh = qk.tile([P, T], F32, name="kh", tag="qk")
                gemm_head(src_q, wqP, bq96, h, qh, use_act=True)
                gemm_head(src_k, wkP, bk96, h, kh, use_act=False)
                pa = pst("pa")
                for jc in range(NT):
                    i0 = jc * P if causal else 0
                    pS = pst("pS")
                    nc.tensor.matmul(
                        pS[:, 0 : T - i0],
                        _r(kh[0:HD, jc * P : (jc + 1) * P]),
                        _r(qh[0:HD, i0:T]),
                        start=True, stop=True,
                    )
                    et = ex.tile([P, T], F32, name="et", tag="ex")
                    if causal:
                        if i0 > 0:
                            nc.gpsimd.tensor_copy(
                                _r(et[:, 0:i0]), zeros_sb[:, 0:i0]
                            )
                        nc.scalar.activation(
                            _r(et[:, i0:T]), pS[:, 0 : T - i0], AF.Exp, scale=SCALE
                        )
                        nc.gpsimd.tensor_mul(
                            _r(et[:, i0 : i0 + P]), et[:, i0 : i0 + P], diag_sb
                        )
                    else:
                        nc.scalar.activation(_r(et), pS[:, :], AF.Exp, scale=SCALE)
                    nc.tensor.matmul(
                        pa[0 : HD + 1, :], _r(Vt[:, jc, h, :]), _r(et),
                        start=(jc == 0), stop=(jc == NT - 1),
                    )
                srow = rows.tile([P, T], F32, name="sumrow", tag="rows")
                nc.vector.reciprocal(srow[HD : HD + 1, :], pa[HD : HD + 1, :])
                srow0 = rows.tile([P, T], F32, name="sumrow0", tag="rows")
                # HW partition_broadcast reads partition 0 only; DMA hop to p0.
                nc.gpsimd.dma_start(out=srow0[0:1, :], in_=srow[HD : HD + 1, :])
                rbc = tmp.tile([P, T], F32, name="rbc", tag="tmp")
                nc.gpsimd.partition_broadcast(rbc, srow0[0:1, :])
                nc.vector.tensor_mul(
                    _r(attnT[0:HD, h, :]), pa[0:HD, :], rbc[0:HD, :]
                )

        def out_proj(attnT, woP_d, bias_sb, dst, residual=None, dst_r=True):
            for e in range(ND):
                wot = wo.tile([HD, H, P], F32, name="wot", tag="wo")
                nc.sync.dma_start(
                    out=_r(wot), in_=_r(woP_d[e].rearrange("k (h ec) -> k h ec", ec=P))
                )
                po = pst("po")
                for h in range(H):
                    nc.tensor.matmul(
                        po, _r(wot[:, h, :]), _r(attnT[0:HD, h, :]),
                        start=(h == 0), stop=(h == H - 1),
                    )
                dslice = dst[:, e, :]
                if dst_r:
                    dslice = _r(dslice)
                if residual is None:
                    nc.vector.tensor_scalar_add(
                        dslice, po, bias_sb[:, e : e + 1]
                    )
                else:
                    nc.vector.scalar_tensor_tensor(
                        dslice, po, bias_sb[:, e : e + 1], residual[:, e, :],
                        ALU.add, ALU.add,
                    )

        def mlp(xn3, x1T, outT_sb):
            ph2 = [
                ps.tile([P, T], F32, name=f"h2_{e}", tag="ps", bufs=8)
                for e in range(ND)
            ]
            for fc in range(NF):
                w1t = wq.tile([P, ND, P], F32, name="w1t", tag="wq")
                nc.sync.dma_start(
                    out=_r(w1t),
                    in_=_r(mW1P_d[fc].rearrange("p (a e) -> p a e", e=P)),
                )
                ph1 = pst("ph1")
                for a in range(ND):
                    nc.tensor.matmul(
                        ph1, _r(w1t[:, a, :]), _r(xn3[:, a, :]),
                        start=(a == 0), stop=(a == ND - 1),
                    )
                yt = tmp.tile([P, T], F32, name="yt", tag="tmp")
                nc.scalar.activation(
                    _r(yt), ph1, AF.Gelu, bias=mB1_sb[:, fc : fc + 1]
                )
                w2t = w2.tile([P, D], F32, name="w2t", tag="w2")
                nc.sync.dma_start(
                    out=_r(w2t), in_=_r(mW2T_d[fc * P : (fc + 1) * P, :])
                )
                for e in range(ND):
                    nc.tensor.matmul(
                        ph2[e], _r(w2t[:, e * P : (e + 1) * P]), _r(yt),
                        start=(fc == 0), stop=(fc == NF - 1),
                    )
            for e in range(ND):
                nc.vector.scalar_tensor_tensor(
                    outT_sb[:, e, :], ph2[e], mB2_sb[:, e : e + 1],
                    x1T[:, e, :], ALU.add, ALU.add,
                )

        def _mark(phase):
            PHASE_MARKS.append((phase, int(nc.get_next_instruction_name()[2:])))

        def body():
            _mark("load_x")
            xT_sb = big.tile([P, ND, T], F32, name="xT_sb", tag="xT")
            nc.sync.dma_start(
                out=_r(xT_sb), in_=_r(xT_d.rearrange("(a p) t -> p a t", p=P))
            )
            _mark("ln0_apply")
            meanb, rstdb = ln_stats(xT_sb)
            xn_s = big.tile([P, ND, T], F32, name="xn_s", tag="xn_s")
            xn_t = big.tile([P, ND, T], F32, name="xn_t", tag="xn_t")
            ln_apply(
                xT_sb, meanb, rstdb,
                [
                    (xn_s, ln_sb["g_s"], ln_sb["b_s"]),
                    (xn_t, ln_sb["g_t"], ln_sb["b_t"]),
                ],
            )

            # --- spatial branch (temporal V is emitted early for overlap) ---
            _mark("sp_v")
            Vt = big.tile([P, NT, H, HD + 1], F32, name="Vt_s", tag="Vt", bufs=2)
            gemm_v_token(xn_s, spWvP_d, spBv_bc, Vt)
            _mark("tp_v")
            Vt2 = big.tile([P, NT, H, HD + 1], F32, name="Vt_t", tag="Vt", bufs=2)
            gemm_v_token(xn_t, tpWvP_d, tpBv_bc, Vt2)
            attnT = big.tile([P, H, T], F32, name="attnT_s", tag="attnT")
            _mark("sp_attn")
            attn_branch(xn_s, xn_s, Vt, attnT, False,
                        spWqP_d, spBq96, spWkP_d, spBk96)
            _mark("sp_oproj")
            soT = big.tile([P, ND, T], F32, name="soT", tag="soT")
            out_proj(attnT, spWoP_d, spBo_sb, soT)

            # --- temporal branch ---
            attnT2 = big.tile([P, H, T], F32, name="attnT_t", tag="attnT")
            _mark("tp_attn")
            attn_branch(xn_t, xn_t, Vt2, attnT2, True,
                        tpWqP_d, tpBq96, tpWkP_d, tpBk96)
            _mark("tp_oproj")
            toT = big.tile([P, ND, T], F32, name="toT", tag="toT")
            out_proj(attnT2, tpWoP_d, tpBo_sb, toT, residual=xn_t)

            # --- cross attention ---
            _mark("cx_v")
            Vt3 = big.tile([P, NT, H, HD + 1], F32, name="Vt_c", tag="Vt", bufs=2)
            gemm_v_token(toT, cWvP_d, cBv_bc, Vt3)
            attnT3 = big.tile([P, H, T], F32, name="attnT_c", tag="attnT")
            _mark("cx_attn")
            attn_branch(soT, toT, Vt3, attnT3, False,
                        cWqP_d, cBq96, cWkP_d, cBk96)
            _mark("cx_oproj")
            x1T = big.tile([P, ND, T], F32, name="x1T", tag="soT")
            out_proj(attnT3, cWoP_d, cBo_sb, x1T, residual=xT_sb)

            # --- MLP ---
            _mark("ln3")
            meanb3, rstdb3 = ln_stats(x1T)
            xn3 = big.tile([P, ND, T], F32, name="xn3", tag="xn_s")
            ln_apply(x1T, meanb3, rstdb3, [(xn3, ln_sb["g_m"], ln_sb["b_m"])])
            outT_sb = big.tile([P, ND, T], F32, name="outT_sb", tag="toT")
            _mark("mlp")
            mlp(xn3, x1T, outT_sb)
            nc.sync.dma_start(
                out=outT_d.rearrange("(a p) t -> p a t", p=P), in_=outT_sb
            )

        if repeat == 1:
            body()
        else:
            with tc.For_i(0, repeat, 1):
                body()

    nc.compile()
    return nc


def _route(inputs):
    """Top-1 expert indices per sample, computed exactly as the reference
    (jax on CPU, f32) — softmax is monotonic so argmax of logits suffices."""
    import jax
    import jax.numpy as jnp

    cpu = jax.devices("cpu")[0]
    with jax.default_device(cpu):
        x = jnp.asarray(inputs["x"])
        h = jax.nn.gelu(
            x.mean(1) @ jnp.asarray(inputs["router_w1"]).T
            + jnp.asarray(inputs["router_b1"]),
            approximate=False,
        )
        logits = (
            h @ jnp.asarray(inputs["router_w2"]).T + jnp.asarray(inputs["router_b2"])
        )
        logits = np.asarray(logits)
    K = logits.shape[1] // 2
    idx_s = np.argmax(logits[:, :K], axis=-1)
    idx_t = np.argmax(logits[:, K:], axis=-1)
    return idx_s, idx_t


_cache = {}


def _get_nc(repeat=1):
    key = ("nc", repeat)
    if key not in _cache:
        _cache[key] = build(repeat=repeat)
    return _cache[key]


def _f(a):
    return np.ascontiguousarray(np.asarray(a), dtype=np.float32)


def _pack_qk(wT_cols):
    # wT_cols: [D, D] = W^T columns for this projection (d, e); e = h*HD+k.
    # -> [H, P, ND*HD] so each head's lhsT tile loads contiguously.
    return _f(
        np.asarray(wT_cols)
        .reshape(ND, P, H, HD)
        .transpose(2, 1, 0, 3)
        .reshape(H, P, ND * HD)
    )


def _pack_v(wT_cols):
    # [D, D] (d, e) -> [ND, P, D]
    return _f(np.asarray(wT_cols).reshape(ND, P, D))


def _pack_wo(w):
    # w: [D, D] (e, d) -> W^T[d, e], d = h*HD+k -> [ND, HD, H*P]
    wt = np.asarray(w).T.reshape(H, HD, ND, P)
    return _f(wt.transpose(2, 1, 0, 3).reshape(ND, HD, H * P))


def _pack_w1(w1):
    # w1: [DFF, D] -> W1^T [D, DFF] -> [NF, P, ND*P]
    w1t = np.asarray(w1).T.reshape(ND, P, NF, P)
    return _f(w1t.transpose(2, 1, 0, 3).reshape(NF, P, ND * P))


def make_in_maps(inputs):
    idx_s, idx_t = _route(inputs)
    ones = np.ones((P, 33), dtype=np.float32)
    diag = np.triu(np.ones((P, P), dtype=np.float32))  # 1 where p <= q
    cWqkvT = np.asarray(inputs["cross_wqkv"]).astype(np.float32).T
    mW2T = _f(np.asarray(inputs["mlp_w2"]).T)
    shared = dict(
        ones=ones,
        diag=diag,
        g_s=_f(inputs["norm_s_g"]),
        b_s=_f(inputs["norm_s_b"]),
        g_t=_f(inputs["norm_t_g"]),
        b_t=_f(inputs["norm_t_b"]),
        g_m=_f(inputs["norm_mlp_g"]),
        b_m=_f(inputs["norm_mlp_b"]),
        cWqP=_pack_qk(cWqkvT[:, 0:D]),
        cWkP=_pack_qk(cWqkvT[:, D : 2 * D]),
        cWvP=_pack_v(cWqkvT[:, 2 * D : 3 * D]),
        cBqkv=_f(inputs["cross_bqkv"]),
        cWoP=_pack_wo(np.asarray(inputs["cross_wo"])),
        cBo=_f(inputs["cross_bo"]),
        mW1P=_pack_w1(np.asarray(inputs["mlp_w1"])),
        mB1=_f(inputs["mlp_b1"]),
        mW2T=mW2T,
        mB2=_f(inputs["mlp_b2"]),
    )
    x = np.asarray(inputs["x"])
    in_maps = []
    for b in range(NCORES):
        s = int(idx_s[b])
        t = int(idx_t[b])
        m = dict(shared)
        m["xT"] = _f(x[b].T)
        spWqkvT = np.asarray(inputs["sp_wqkv"])[s].astype(np.float32).T
        m["spWqP"] = _pack_qk(spWqkvT[:, 0:D])
        m["spWkP"] = _pack_qk(spWqkvT[:, D : 2 * D])
        m["spWvP"] = _pack_v(spWqkvT[:, 2 * D : 3 * D])
        m["spBqkv"] = _f(np.asarray(inputs["sp_bqkv"])[s])
        m["spWoP"] = _pack_wo(np.asarray(inputs["sp_wo"])[s])
        m["spBo"] = _f(np.asarray(inputs["sp_bo"])[s])
        m["tpWqP"] = _pack_qk(np.asarray(inputs["tp_wq"])[t].astype(np.float32).T)
        m["tpBq"] = _f(np.asarray(inputs["tp_bq"])[t])
        m["tpWkP"] = _pack_qk(np.asarray(inputs["tp_wk"])[t].astype(np.float32).T)
        m["tpBk"] = _f(np.asarray(inputs["tp_bk"])[t])
        m["tpWvP"] = _pack_v(np.asarray(inputs["tp_wv"])[t].astype(np.float32).T)
        m["tpBv"] = _f(np.asarray(inputs["tp_bv"])[t])
        m["tpWoP"] = _pack_wo(np.asarray(inputs["tp_wo"])[t])
        m["tpBo"] = _f(np.asarray(inputs["tp_bo"])[t])
        in_maps.append(m)
    return in_maps


def kernel(**inputs) -> np.ndarray:
    repeat = int(os.environ.get("KREPEAT", "1"))
    nc = _get_nc(repeat=repeat)
    in_maps = make_in_maps(inputs)
    res = bass_utils.run_bass_kernel_spmd(nc, in_maps, core_ids=list(range(NCORES)))
    out = np.stack(
        [np.ascontiguousarray(res.results[b]["outT"].T) for b in range(NCORES)]
    )
    return out



# revision 7
# speedup vs baseline: 1.3761x; 1.3761x over previous
"""Trainium2 Bass kernel for nn_MixtureOfRoutingAttention.

Strategy: data-parallel over B=8 (one sample per NeuronCore). The top-1
routing argmax is computed on host (it only decides dispatch); the selected
expert weight stacks are gathered per sample on host, packed into large
contiguous bf16 blocks, and shipped pre-transposed to each core. Everything
x-dependent (LayerNorms, QKV/out projections, three attentions, MLP) runs on
device with bf16 matmul operands and fp32 PSUM accumulation.

Device-side layout: activations are kept feature-major (x^T: [D, T], feature
on partitions) so every GEMM contracts over the partition axis with zero
on-chip transposes. Attention uses transposed scores S^T[j, i] so softmax
normalization is deferred: a ones-column prepended to V yields the softmax
denominators as row 0 of the PV psum accumulation (partition 0, so it feeds
partition_broadcast directly). The temporal branch computes only the lower
triangle of S^T and the matching PV blocks. LayerNorm rstd is computed as
exp(-0.5*ln(var+eps)) so the whole kernel needs only the natural_log_exp
and gelu ACT table sets (2 table loads per iteration instead of 4).
"""

import math
import os
from contextlib import ExitStack

import numpy as np
import ml_dtypes

import concourse.bass as bass
import concourse.bacc as bacc
import concourse.tile as tile
import concourse.mybir as mybir
from concourse import bass_utils

dt = mybir.dt
AF = mybir.ActivationFunctionType
ALU = mybir.AluOpType

P = 128
T = 512
D = 768
H = 8
HD = 96
DFF = 3072
NCORES = 8
ND = D // P  # 6
NT = T // P  # 4
NF = DFF // P  # 24
FCC = 4  # mlp fc-chunk (DMA granularity)
SCALE = 1.0 / math.sqrt(HD)
EPS = 1e-5
F32 = dt.float32
BF = dt.bfloat16
NPBF = ml_dtypes.bfloat16


def _bcast_ap(src_1d, parts=P):
    """Partition-broadcast DMA access pattern for a 1-D DRAM AP."""
    return bass.AP(
        tensor=src_1d.tensor,
        offset=src_1d.offset,
        ap=[[0, parts], list(src_1d.ap[0])],
    )


PHASE_MARKS = []


def build(repeat: int = 1):
    PHASE_MARKS.clear()
    nc = bacc.Bacc(
        "TRN2",
        target_bir_lowering=False,
        debug=False,
        enable_asserts=False,
        num_devices=NCORES,
    )

    def din(name, shape, dtype=BF):
        return nc.dram_tensor(name, shape, dtype, kind="ExternalInput").ap()

    xT_d = din("xT", [D, T])
    diag_d = din("diag", [P, P])
    ln_d = {n: din(n, [D], F32) for n in ("g_s", "b_s", "g_t", "b_t", "g_m", "b_m")}

    wqk_d = {b: din(f"{b}Wqk", [P, 2 * H * ND * HD]) for b in ("sp", "tp", "cx")}
    wv_d = {b: din(f"{b}Wv", [P, ND * D]) for b in ("sp", "tp", "cx")}
    wo_d = {b: din(f"{b}Wo", [HD, ND * H * P]) for b in ("sp", "tp", "cx")}
    bq_d = {b: din(f"{b}Bq", [D], F32) for b in ("sp", "tp", "cx")}
    bk_d = {b: din(f"{b}Bk", [D], F32) for b in ("sp", "tp", "cx")}
    bv_d = {b: din(f"{b}Bv", [D], F32) for b in ("sp", "tp", "cx")}
    bo_d = {b: din(f"{b}Bo", [D], F32) for b in ("sp", "tp", "cx")}

    mW1_d = din("mW1", [P, NF * ND * P])
    mB1_d = din("mB1", [DFF], F32)
    mW2_d = din("mW2", [P, NF * D])
    mB2_d = din("mB2", [D], F32)

    outT_d = nc.dram_tensor("outT", [D, T], BF, kind="ExternalOutput").ap()

    with tile.TileContext(nc) as tc, ExitStack() as ctx:
        ctx.enter_context(
            nc.allow_low_precision(reason="bf16 matmul operands, fp32 accumulation")
        )
        const = ctx.enter_context(tc.tile_pool(name="const", bufs=1))
        big = ctx.enter_context(tc.tile_pool(name="big", bufs=1))
        wqkp = ctx.enter_context(tc.tile_pool(name="wqkp", bufs=2))
        wvp = ctx.enter_context(tc.tile_pool(name="wvp", bufs=2))
        wop = ctx.enter_context(tc.tile_pool(name="wop", bufs=1))
        w1p = ctx.enter_context(tc.tile_pool(name="w1p", bufs=2))
        w2p = ctx.enter_context(tc.tile_pool(name="w2p", bufs=2))
        tmp = ctx.enter_context(tc.tile_pool(name="tmp", bufs=4))
        ex = ctx.enter_context(tc.tile_pool(name="ex", bufs=5))
        qk = ctx.enter_context(tc.tile_pool(name="qk", bufs=6))
        rows = ctx.enter_context(tc.tile_pool(name="rows", bufs=6))
        ps = ctx.enter_context(tc.tile_pool(name="ps", bufs=8, space="PSUM"))

        def pst(nm):
            return ps.tile([P, T], F32, name=nm, tag="ps", bufs=8)

        # ---- constants / params (loaded once, outside any repeat loop) ----
        ones_bf = const.tile([P, 1], BF, name="ones_bf", tag="ones")
        nc.vector.memset(ones_bf, 1.0)
        diag_sb = const.tile([P, P], BF, name="diag_sb", tag="diag")
        nc.sync.dma_start(out=diag_sb, in_=diag_d)
        epsc = const.tile([P, 1], F32, name="epsc", tag="eps")
        nc.vector.memset(epsc, EPS)

        ln_sb = {}
        for n in ln_d:
            t = const.tile([P, ND], F32, name=f"ln_{n}", tag=f"ln_{n}")
            nc.sync.dma_start(out=t, in_=ln_d[n].rearrange("(a p) -> p a", p=P))
            ln_sb[n] = t

        bq96, bk96, vbias, bo_sb = {}, {}, {}, {}
        for b in ("sp", "tp", "cx"):
            t = const.tile([HD, H], F32, name=f"bq96_{b}", tag=f"bq96_{b}")
            nc.sync.dma_start(out=t, in_=bq_d[b].rearrange("(h k) -> k h", k=HD))
            bq96[b] = t
            t = const.tile([HD, H], F32, name=f"bk96_{b}", tag=f"bk96_{b}")
            nc.sync.dma_start(out=t, in_=bk_d[b].rearrange("(h k) -> k h", k=HD))
            bk96[b] = t
            t = const.tile([P, D], F32, name=f"vb_{b}", tag=f"vb_{b}")
            nc.gpsimd.dma_start(out=t, in_=_bcast_ap(bv_d[b]))
            vbias[b] = t
            t = const.tile([P, ND], F32, name=f"bo_{b}", tag=f"bo_{b}")
            nc.sync.dma_start(out=t, in_=bo_d[b].rearrange("(a p) -> p a", p=P))
            bo_sb[b] = t

        mB1_sb = const.tile([P, NF], F32, name="mB1_sb", tag="mB1")
        nc.sync.dma_start(out=mB1_sb, in_=mB1_d.rearrange("(a p) -> p a", p=P))
        mB2_sb = const.tile([P, ND], F32, name="mB2_sb", tag="mB2")
        nc.sync.dma_start(out=mB2_sb, in_=mB2_d.rearrange("(a p) -> p a", p=P))

        # ---- body helpers ----

        def ln_stats(src):
            """src: [P, ND, T] bf16. Returns (meanb, rstdb) [P, T] bf16."""
            ps_m = pst("ps_m")
            ps_s = pst("ps_s")
            for a in range(ND):
                sq = tmp.tile([P, T], BF, name="sq", tag="tmp")
                nc.scalar.square(sq, src[:, a, :])
                nc.tensor.matmul(
                    ps_m[0:1, :], ones_bf, src[:, a, :],
                    start=(a == 0), stop=(a == ND - 1),
                )
                nc.tensor.matmul(
                    ps_s[0:1, :], ones_bf, sq,
                    start=(a == 0), stop=(a == ND - 1),
                )
            mrow = rows.tile([1, T], F32, name="mrow", tag="rows")
            nc.vector.tensor_scalar_mul(mrow, ps_m[0:1, :], 1.0 / D)
            srow = rows.tile([1, T], F32, name="srow", tag="rows")
            nc.vector.tensor_scalar_mul(srow, ps_s[0:1, :], 1.0 / D)
            trow = rows.tile([1, T], F32, name="trow", tag="rows")
            nc.vector.tensor_mul(trow, mrow, mrow)
            # var = E[x^2] - m^2 ; rstd = exp(-0.5*ln(var+eps))
            nc.vector.tensor_sub(srow, srow, trow)
            urow = rows.tile([1, T], F32, name="urow", tag="rows")
            nc.scalar.activation(urow, srow, AF.Ln, bias=epsc[0:1, :])
            rrow = rows.tile([1, T], BF, name="rrow", tag="rows")
            nc.scalar.activation(rrow, urow, AF.Exp, scale=-0.5)
            mrow_bf = rows.tile([1, T], BF, name="mrow_bf", tag="rows")
            nc.vector.tensor_copy(mrow_bf, mrow)

            meanb = big.tile([P, T], BF, name="meanb", tag="meanb", bufs=2)
            nc.gpsimd.partition_broadcast(meanb, mrow_bf)
            rstdb = big.tile([P, T], BF, name="rstdb", tag="rstdb", bufs=2)
            nc.gpsimd.partition_broadcast(rstdb, rrow)
            return meanb, rstdb

        def ln_apply(src, meanb, rstdb, outs):
            """outs: list of (dst [P, ND, T] bf16, gamma_sb, beta_sb)."""
            for a in range(ND):
                xc = tmp.tile([P, T], BF, name="xc", tag="tmp")
                nc.vector.tensor_sub(xc, src[:, a, :], meanb)
                nc.vector.tensor_mul(xc, xc, rstdb)
                for dst, g_sb, b_sb in outs:
                    nc.scalar.activation(
                        dst[:, a, :], xc, AF.Identity,
                        bias=b_sb[:, a : a + 1], scale=g_sb[:, a : a + 1],
                    )

        def gemm_head(src, wqk_sb, qki, bias96, h, dst96, use_act=False):
            """dst96[0:HD, :] = (W[:, head h cols].T @ src) + bias."""
            pq = pst("pq")
            for a in range(ND):
                nc.tensor.matmul(
                    pq[0:HD, :], wqk_sb[:, qki, h, a, :], src[:, a, :],
                    start=(a == 0), stop=(a == ND - 1),
                )
            if use_act:
                nc.scalar.activation(
                    dst96[0:HD, :], pq[0:HD, :], AF.Identity,
                    bias=bias96[:, h : h + 1],
                )
            else:
                nc.vector.tensor_scalar_add(
                    dst96[0:HD, :], pq[0:HD, :], bias96[:, h : h + 1]
                )

        def gemm_v_token(src, wv_sb, vbias_bc, Vt):
            """Vt: [P, NT, H, HD+1] token-major V with trailing ones column."""
            nc.vector.memset(Vt[:, :, :, HD], 1.0)
            for half, n in ((0, 512), (1, 256)):
                pvs = [pst(f"pv{t}") for t in range(NT)]
                for a in range(ND):
                    for t in range(NT):
                        nc.tensor.matmul(
                            pvs[t][:, 0:n],
                            src[:, a, t * P : (t + 1) * P],
                            wv_sb[:, a, half * 512 : half * 512 + n],
                            start=(a == 0), stop=(a == ND - 1),
                        )
                for t in range(NT):
                    if half == 0:
                        nc.vector.tensor_add(
                            Vt[:, t, 0:5, 0:HD],
                            pvs[t][:, 0:480].rearrange("p (h k) -> p h k", k=HD),
                            vbias_bc[:, 0:480].rearrange("p (h k) -> p h k", k=HD),
                        )
                        nc.vector.tensor_add(
                            Vt[:, t, 5, 0:32],
                            pvs[t][:, 480:512],
                            vbias_bc[:, 480:512],
                        )
                    else:
                        nc.vector.tensor_add(
                            Vt[:, t, 5, 32:HD],
                            pvs[t][:, 0:64],
                            vbias_bc[:, 512:576],
                        )
                        nc.vector.tensor_add(
                            Vt[:, t, 6:8, 0:HD],
                            pvs[t][:, 64:256].rearrange("p (h k) -> p h k", k=HD),
                            vbias_bc[:, 576:768].rearrange("p (h k) -> p h k", k=HD),
                        )

        def attn_head_core(qh, kh, Vt, attnT, causal, h):
            """Scores, exp, PV, deferred-softmax normalization for one head."""
            ets = []
            for jc in range(NT):
                i0 = jc * P if causal else 0
                pS = pst("pS")
                nc.tensor.matmul(
                    pS[:, 0 : T - i0],
                    kh[0:HD, jc * P : (jc + 1) * P],
                    qh[0:HD, i0:T],
                    start=True, stop=True,
                )
                et = ex.tile([P, T], BF, name="et", tag="ex")
                nc.scalar.activation(
                    et[:, i0:T], pS[:, 0 : T - i0], AF.Exp, scale=SCALE
                )
                if causal:
                    nc.gpsimd.tensor_mul(
                        et[:, i0 : i0 + P], et[:, i0 : i0 + P], diag_sb
                    )
                ets.append(et)
            pa = pst("pa")
            for jc in range(NT):
                i0 = jc * P if causal else 0
                # jc=0 covers the full psum row (start lazily zeroes the whole
                # 2KB zero region); later jc's accumulate only their causal
                # suffix [jc*P:T].
                nc.tensor.matmul(
                    pa[0 : HD + 1, i0:T], Vt[:, jc, h, :], ets[jc][:, i0:T],
                    start=(jc == 0), stop=(jc == NT - 1),
                )
            srow = rows.tile([HD + 1, T], BF, name="sumrow", tag="srow", bufs=3)
            nc.vector.reciprocal(srow[HD : HD + 1, :], pa[HD : HD + 1, :])
            # Replicating SBUF->SBUF DMA: broadcast the reciprocal row
            # (partition HD) to all partitions via a stride-0 middle dim.
            s = srow[HD : HD + 1, :]
            rbc = tmp.tile([P, T], BF, name="rbc", tag="tmp")
            nc.sync.dma_start(
                out=rbc,
                in_=bass.AP(
                    tensor=s.tensor, offset=s.offset,
                    ap=[list(s.ap[0]), [0, P], list(s.ap[-1])],
                ),
            )
            nc.vector.tensor_mul(
                attnT[0:HD, h, :], pa[0:HD, :], rbc[0:HD, :]
            )

        def attn_branch(src_q, src_k, Vt, attnT, causal, wqk_sb, bq, bk):
            """Per-head q/k projection software-pipelined with attention."""
            qs, ks = [None] * H, [None] * H
            for h in range(H):
                qs[h] = qk.tile([P, T], BF, name="qh", tag="qk")
                ks[h] = qk.tile([P, T], BF, name="kh", tag="qk")
                gemm_head(src_q, wqk_sb, 0, bq, h, qs[h], use_act=True)
                gemm_head(src_k, wqk_sb, 1, bk, h, ks[h], use_act=False)
                if h > 0:
                    attn_head_core(qs[h - 1], ks[h - 1], Vt, attnT, causal, h - 1)
                    qs[h - 1] = ks[h - 1] = None
            attn_head_core(qs[H - 1], ks[H - 1], Vt, attnT, causal, H - 1)

        def out_proj(attnT, wo_sb, bias_sb, dst, residual=None):
            for e in range(ND):
                po = pst("po")
                for h in range(H):
                    nc.tensor.matmul(
                        po, wo_sb[:, e, h, :], attnT[0:HD, h, :],
                        start=(h == 0), stop=(h == H - 1),
                    )
                if residual is None:
                    nc.vector.tensor_scalar_add(
                        dst[:, e, :], po, bias_sb[:, e : e + 1]
                    )
                else:
                    nc.vector.scalar_tensor_tensor(
                        dst[:, e, :], po, bias_sb[:, e : e + 1],
                        residual[:, e, :], ALU.add, ALU.add,
                    )

        def load_wqk(b):
            t = wqkp.tile([P, 2, H, ND, HD], BF, name=f"wqk_{b}", tag="wqk")
            nc.sync.dma_start(
                out=t,
                in_=wqk_d[b].rearrange("p (q h a k) -> p q h a k", q=2, h=H, a=ND),
            )
            return t

        def load_wv(b):
            t = wvp.tile([P, ND, D], BF, name=f"wv_{b}", tag="wv")
            nc.sync.dma_start(
                out=t, in_=wv_d[b].rearrange("p (a e) -> p a e", a=ND)
            )
            return t

        def load_wo(b):
            t = wop.tile([HD, ND, H, P], BF, name=f"wo_{b}", tag="wo")
            nc.sync.dma_start(
                out=t, in_=wo_d[b].rearrange("k (e h ec) -> k e h ec", e=ND, h=H)
            )
            return t

        def mlp(xn3, x1T, outT_sb):
            ph2 = [
                ps.tile([P, T], F32, name=f"h2_{e}", tag="ps", bufs=8)
                for e in range(ND)
            ]
            for c in range(NF // FCC):
                w1t = w1p.tile([P, FCC, ND, P], BF, name="w1t", tag="w1")
                nc.sync.dma_start(
                    out=w1t,
                    in_=mW1_d.rearrange("p (f a e) -> p f a e", f=NF, a=ND)[
                        :, c * FCC : (c + 1) * FCC, :, :
                    ],
                )
                w2t = w2p.tile([P, FCC, D], BF, name="w2t", tag="w2")
                nc.sync.dma_start(
                    out=w2t,
                    in_=mW2_d.rearrange("p (f e) -> p f e", f=NF)[
                        :, c * FCC : (c + 1) * FCC, :
                    ],
                )
                for j in range(FCC):
                    fc = c * FCC + j
                    ph1 = pst("ph1")
                    for a in range(ND):
                        nc.tensor.matmul(
                            ph1, w1t[:, j, a, :], xn3[:, a, :],
                            start=(a == 0), stop=(a == ND - 1),
                        )
                    yt = tmp.tile([P, T], BF, name="yt", tag="tmp")
                    nc.scalar.activation(
                        yt, ph1, AF.Gelu, bias=mB1_sb[:, fc : fc + 1]
                    )
                    for e in range(ND):
                        nc.tensor.matmul(
                            ph2[e], w2t[:, j, e * P : (e + 1) * P], yt,
                            start=(fc == 0), stop=(fc == NF - 1),
                        )
            for e in range(ND):
                nc.vector.scalar_tensor_tensor(
                    outT_sb[:, e, :], ph2[e], mB2_sb[:, e : e + 1],
                    x1T[:, e, :], ALU.add, ALU.add,
                )

        def _mark(phase):
            PHASE_MARKS.append((phase, int(nc.get_next_instruction_name()[2:])))

        def body():
            _mark("load_x")
            xT_sb = big.tile([P, ND, T], BF, name="xT_sb", tag="xT")
            nc.sync.dma_start(
                out=xT_sb, in_=xT_d.rearrange("(a p) t -> p a t", p=P)
            )
            _mark("ln0")
            meanb, rstdb = ln_stats(xT_sb)
            xn_s = big.tile([P, ND, T], BF, name="xn_s", tag="xn_s")
            xn_t = big.tile([P, ND, T], BF, name="xn_t", tag="xn_t")
            ln_apply(
                xT_sb, meanb, rstdb,
                [
                    (xn_s, ln_sb["g_s"], ln_sb["b_s"]),
                    (xn_t, ln_sb["g_t"], ln_sb["b_t"]),
                ],
            )

            # --- spatial branch (temporal V is emitted early for overlap) ---
            _mark("sp_v")
            wv_sp = load_wv("sp")
            Vt = big.tile([P, NT, H, HD + 1], BF, name="Vt_s", tag="Vt", bufs=2)
            gemm_v_token(xn_s, wv_sp, vbias["sp"], Vt)
            _mark("tp_v")
            wv_tp = load_wv("tp")
            Vt2 = big.tile([P, NT, H, HD + 1], BF, name="Vt_t", tag="Vt", bufs=2)
            gemm_v_token(xn_t, wv_tp, vbias["tp"], Vt2)
            attnT = big.tile([P, H, T], BF, name="attnT_s", tag="attnT", bufs=2)
            _mark("sp_attn")
            wqk_sp = load_wqk("sp")
            attn_branch(xn_s, xn_s, Vt, attnT, False, wqk_sp, bq96["sp"], bk96["sp"])
            _mark("sp_oproj")
            wo_sp = load_wo("sp")
            soT = big.tile([P, ND, T], BF, name="soT", tag="soT")
            out_proj(attnT, wo_sp, bo_sb["sp"], soT)

            # --- temporal branch ---
            attnT2 = big.tile([P, H, T], BF, name="attnT_t", tag="attnT", bufs=2)
            _mark("tp_attn")
            wqk_tp = load_wqk("tp")
            attn_branch(xn_t, xn_t, Vt2, attnT2, True, wqk_tp, bq96["tp"], bk96["tp"])
            _mark("tp_oproj")
            wo_tp = load_wo("tp")
            toT = big.tile([P, ND, T], BF, name="toT", tag="toT")
            out_proj(attnT2, wo_tp, bo_sb["tp"], toT, residual=xn_t)

            # --- cross attention ---
            _mark("cx_v")
            wv_cx = load_wv("cx")
            Vt3 = big.tile([P, NT, H, HD + 1], BF, name="Vt_c", tag="Vt", bufs=2)
            gemm_v_token(toT, wv_cx, vbias["cx"], Vt3)
            attnT3 = big.tile([P, H, T], BF, name="attnT_c", tag="attnT", bufs=2)
            _mark("cx_attn")
            wqk_cx = load_wqk("cx")
            attn_branch(soT, toT, Vt3, attnT3, False, wqk_cx, bq96["cx"], bk96["cx"])
            _mark("cx_oproj")
            wo_cx = load_wo("cx")
            x1T = big.tile([P, ND, T], BF, name="x1T", tag="x1T")
            out_proj(attnT3, wo_cx, bo_sb["cx"], x1T, residual=xT_sb)

            # --- MLP ---
            _mark("ln3")
            meanb3, rstdb3 = ln_stats(x1T)
            xn3 = big.tile([P, ND, T], BF, name="xn3", tag="xn_s")
            ln_apply(x1T, meanb3, rstdb3, [(xn3, ln_sb["g_m"], ln_sb["b_m"])])
            outT_sb = big.tile([P, ND, T], BF, name="outT_sb", tag="toT")
            _mark("mlp")
            mlp(xn3, x1T, outT_sb)
            nc.sync.dma_start(
                out=outT_d.rearrange("(a p) t -> p a t", p=P), in_=outT_sb
            )

        if repeat == 1:
            body()
        else:
            with tc.For_i(0, repeat, 1):
                body()

    nc.compile()
    return nc


def _route(inputs):
    """Top-1 expert indices per sample, computed exactly as the reference
    (jax on CPU, f32) — softmax is monotonic so argmax of logits suffices."""
    import jax
    import jax.numpy as jnp

    cpu = jax.devices("cpu")[0]
    with jax.default_device(cpu):
        x = jnp.asarray(inputs["x"])
        h = jax.nn.gelu(
            x.mean(1) @ jnp.asarray(inputs["router_w1"]).T
            + jnp.asarray(inputs["router_b1"]),
            approximate=False,
        )
        logits = (
            h @ jnp.asarray(inputs["router_w2"]).T + jnp.asarray(inputs["router_b2"])
        )
        logits = np.asarray(logits)
    K = logits.shape[1] // 2
    idx_s = np.argmax(logits[:, :K], axis=-1)
    idx_t = np.argmax(logits[:, K:], axis=-1)
    return idx_s, idx_t


_cache = {}


def _get_nc(repeat=1):
    key = ("nc", repeat)
    if key not in _cache:
        _cache[key] = build(repeat=repeat)
    return _cache[key]


def _f(a):
    return np.ascontiguousarray(np.asarray(a), dtype=np.float32)


def _bf(a):
    return np.ascontiguousarray(np.asarray(a, dtype=np.float32).astype(NPBF))


def _pack_qk_pair(wqT, wkT):
    # wqT/wkT: [D, D] = W^T columns (d, e); e = h*HD+k.
    # -> [P, 2*H*ND*HD] so the whole q/k weight pair is one contiguous DMA.
    arr = np.stack([np.asarray(wqT), np.asarray(wkT)])  # [2, D, D]
    arr = arr.reshape(2, ND, P, H, HD).transpose(2, 0, 3, 1, 4)
    return _bf(arr.reshape(P, 2 * H * ND * HD))


def _pack_v(wT):
    # [D, D] (d, e) -> [P, ND*D]
    return _bf(np.asarray(wT).reshape(ND, P, D).transpose(1, 0, 2).reshape(P, ND * D))


def _pack_wo(w):
    # w: [D, D] (e, d) -> W^T[d, e], d = h*HD+k -> [HD, ND*H*P]
    wt = np.asarray(w).T.reshape(H, HD, ND, P)
    return _bf(wt.transpose(1, 2, 0, 3).reshape(HD, ND * H * P))


def _pack_w1(w1):
    # w1: [DFF, D] -> W1^T [D, DFF] -> [P, NF*ND*P]
    w1t = np.asarray(w1).T.reshape(ND, P, NF, P)
    return _bf(w1t.transpose(1, 2, 0, 3).reshape(P, NF * ND * P))


def _pack_w2(w2):
    # w2: [D, DFF] -> W2^T [DFF, D] -> [P, NF*D]
    w2t = np.asarray(w2).T.reshape(NF, P, D)
    return _bf(w2t.transpose(1, 0, 2).reshape(P, NF * D))


def make_in_maps(inputs):
    idx_s, idx_t = _route(inputs)
    diag = np.triu(np.ones((P, P), dtype=np.float32))  # 1 where p <= q
    cWqkvT = np.asarray(inputs["cross_wqkv"]).astype(np.float32).T
    cb = _f(inputs["cross_bqkv"])
    shared = dict(
        diag=_bf(diag),
        g_s=_f(inputs["norm_s_g"]),
        b_s=_f(inputs["norm_s_b"]),
        g_t=_f(inputs["norm_t_g"]),
        b_t=_f(inputs["norm_t_b"]),
        g_m=_f(inputs["norm_mlp_g"]),
        b_m=_f(inputs["norm_mlp_b"]),
        cxWqk=_pack_qk_pair(cWqkvT[:, 0:D], cWqkvT[:, D : 2 * D]),
        cxWv=_pack_v(cWqkvT[:, 2 * D : 3 * D]),
        cxWo=_pack_wo(np.asarray(inputs["cross_wo"])),
        cxBq=cb[0:D],
        cxBk=cb[D : 2 * D],
        cxBv=cb[2 * D : 3 * D],
        cxBo=_f(inputs["cross_bo"]),
        mW1=_pack_w1(np.asarray(inputs["mlp_w1"])),
        mB1=_f(inputs["mlp_b1"]),
        mW2=_pack_w2(np.asarray(inputs["mlp_w2"])),
        mB2=_f(inputs["mlp_b2"]),
    )
    x = np.asarray(inputs["x"])
    in_maps = []
    for b in range(NCORES):
        s = int(idx_s[b])
        t = int(idx_t[b])
        m = dict(shared)
        m["xT"] = _bf(x[b].T)
        spWqkvT = np.asarray(inputs["sp_wqkv"])[s].astype(np.float32).T
        spb = _f(np.asarray(inputs["sp_bqkv"])[s])
        m["spWqk"] = _pack_qk_pair(spWqkvT[:, 0:D], spWqkvT[:, D : 2 * D])
        m["spWv"] = _pack_v(spWqkvT[:, 2 * D : 3 * D])
        m["spWo"] = _pack_wo(np.asarray(inputs["sp_wo"])[s])
        m["spBq"] = spb[0:D]
        m["spBk"] = spb[D : 2 * D]
        m["spBv"] = spb[2 * D : 3 * D]
        m["spBo"] = _f(np.asarray(inputs["sp_bo"])[s])
        m["tpWqk"] = _pack_qk_pair(
            np.asarray(inputs["tp_wq"])[t].astype(np.float32).T,
            np.asarray(inputs["tp_wk"])[t].astype(np.float32).T,
        )
        m["tpWv"] = _pack_v(np.asarray(inputs["tp_wv"])[t].astype(np.float32).T)
        m["tpWo"] = _pack_wo(np.asarray(inputs["tp_wo"])[t])
        m["tpBq"] = _f(np.asarray(inputs["tp_bq"])[t])
        m["tpBk"] = _f(np.asarray(inputs["tp_bk"])[t])
        m["tpBv"] = _f(np.asarray(inputs["tp_bv"])[t])
        m["tpBo"] = _f(np.asarray(inputs["tp_bo"])[t])
        in_maps.append(m)
    return in_maps


def kernel(**inputs) -> np.ndarray:
    repeat = int(os.environ.get("KREPEAT", "1"))
    nc = _get_nc(repeat=repeat)
    in_maps = make_in_maps(inputs)
    res = bass_utils.run_bass_kernel_spmd(nc, in_maps, core_ids=list(range(NCORES)))
    out = np.stack(
        [
            np.ascontiguousarray(
                np.asarray(res.results[b]["outT"], dtype=np.float32).T
            )
            for b in range(NCORES)
        ]
    )
    return out
